# revision 14
# baseline (speedup 1.0000x reference)
"""TRN2 Bass kernel for nn_BiDecoder (GNN edge rating decoder), 8 NeuronCores.

ratings[e] = sum_r softmax_r(ufeat[src[e]] @ Ps[r] @ ifeat[dst[e]]) * (r+1)

v2: item-sorted edges + per-item transformed features -> TensorE does the
bilinear contraction, eliminating the v1 per-edge DVE mul/reduce (366us DVE
+ 325us ACT busy in the v1 trace).

Host: sort edges by dst (item), cut into 8 equal per-core runs, greedy-pack
sorted edges into 128-edge tiles with at most S=8 distinct items per tile.
Precompute Yt[v] = Ps[r] @ ifeat[v] (R*D per item) once; per tile send the
8 items' Yt as a (64, R*S=40) fp16 rhs pack, the gathered user features as
a (64, 128) fp16 stationary, and a (128, 8) one-hot slot mask.

v4 (PE shape): v2 ran one K=64 matmul per tile; the 128-col LDWEIGHTS and
the 40-col MATMUL serialized at the 1.2 GHz mid p-state (106+197 ns/tile,
measured). tile_position row-tiling to overlap them faults on HW (CoreSim
passes, device errors), so instead each PAIR of tiles becomes ONE full
K=128 matmul: stationary = [usT_even; usT_odd] (128x128), moving = block
diagonal [[Yt_even, 0], [0, Yt_odd]] (128x80). The zero blocks are memset
once in SBUF (the 3 pool slots) and only the nonzero halves are re-DMAd,
so PSUM gets Z[e, t, r, j] exactly as in v2 at half the LDWEIGHTS and
double the per-matmul stream.

Z lands in PSUM (128, 5, 8) per tile; ACT drains 6-pair PSUM banks to
fp16, DVE masks (one-hot slot mul) + 3-level tree over the 8 slots -> true
scores, then a per-block softmax-weighted rating tail.

v4 (DMA shape): the HWDGE queue costs ~625ns per DMA instruction
regardless of size (273 DMAs = 171us serialized in the v2 trace), so the
ust + mask streams are concatenated per block into ONE dram tensor: 3 DMA
instructions per 64-tile block (stream, ytd top half, ytd bottom half).
"""
import sys

sys.path.insert(0, "/opt/trn_rl_repo")
import numpy as np

P = 128
D = 64
R = 5
S = 8  # item slots per tile
W = R * S  # per-tile score columns
W2 = 2 * W  # matmul N per tile-pair
GP = 6  # tile-pairs per PSUM bank group (6*80*4B = 1920B <= 2KB bank)
N_USERS, N_ITEMS, E = 100000, 50000, 1000000
N_CORES = 8
E_CORE = E // N_CORES
BLOCK_TILES = [64] * 15 + [32]  # 992 tiles = 126976 edge slots (~1.6% slack)
N_TILES = sum(BLOCK_TILES)
PAD_E = N_TILES * P
# per-block fused stream: [usT pairs (nt/2 * 128 cols) | mask (nt * 8 cols)]
BLOCK_STREAM_COLS = [nt // 2 * P + nt * S for nt in BLOCK_TILES]
STREAM_COLS = sum(BLOCK_STREAM_COLS)

_NC_CACHE = {}


def _build_kernel():
    import concourse.bacc as bacc
    import concourse.mybir as mybir
    import concourse.tile as tile

    nc = bacc.Bacc(None, target_bir_lowering=False)
    f32, f16 = mybir.dt.float32, mybir.dt.float16

    str_d = nc.dram_tensor("stream", [P, STREAM_COLS], f16, kind="ExternalInput")
    ytd_d = nc.dram_tensor("ytd", [P, N_TILES // 2 * W], f16, kind="ExternalInput")
    vals_d = nc.dram_tensor("vals", [P, R], f32, kind="ExternalInput")
    out_d = nc.dram_tensor("out", [P, N_TILES], f32, kind="ExternalOutput")

    X = mybir.AxisListType.X
    ADD = mybir.AluOpType.add
    TMAX = max(BLOCK_TILES)
    QMAX = TMAX // 2

    with tile.TileContext(nc) as tc:
        with nc.allow_low_precision(reason="rel tol 2e-2; fp16 scores are fine"):
            with (
                tc.tile_pool(name="const", bufs=1) as cpool,
                tc.tile_pool(name="st", bufs=3) as stpool,
                tc.tile_pool(name="yt", bufs=1) as ypool,
                tc.tile_pool(name="zpsumA", bufs=3, space="PSUM") as zpoolA,
                tc.tile_pool(name="zpsumB", bufs=3, space="PSUM") as zpoolB,
                tc.tile_pool(name="zh", bufs=2) as zhpool,
                tc.tile_pool(name="prod", bufs=2) as ppool,
                tc.tile_pool(name="sc", bufs=2) as spool,
                tc.tile_pool(name="tail", bufs=2) as tpool,
            ):
                vals_t = cpool.tile([P, R], f32)
                nc.sync.dma_start(vals_t[:], vals_d[:])

                # persistent manually-rotated ytb buffers: the off-diagonal
                # zero blocks are memset ONCE and survive across blocks
                # (a pool slot would flag cross-generation reads)
                ybufs = []
                for i in range(3):
                    yb = ypool.tile([P, QMAX * W2], f16, tag=f"ytP{i}")
                    y4 = yb[:].rearrange("p (q x) -> p q x", q=QMAX)
                    nc.vector.memset(y4[0:D, :, W:W2], 0.0)
                    nc.vector.memset(y4[D:P, :, 0:W], 0.0)
                    ybufs.append(yb)

                toff = 0  # running tile offset
                soff = 0  # running stream col offset
                for bi, nt in enumerate(BLOCK_TILES):
                    first = bi == 0
                    nq = nt // 2
                    stb = stpool.tile([P, QMAX * P + TMAX * S], f16, tag="st")
                    ytb = ybufs[bi % 3]
                    yt4 = ytb[:].rearrange("p (q x) -> p q x", q=QMAX)

                    # fused [usT | mask] stream: 1 DMA (split for pipeline
                    # fill on the first block); ytd: 2 strided DMAs into the
                    # block-diagonal layout
                    bounds = [0, 4, 16] if first else [0]
                    bounds = sorted(set(bounds + [nt]))
                    for c0, c1 in zip(bounds, bounds[1:]):
                        nc.sync.dma_start(
                            stb[:, c0 // 2 * P : c1 // 2 * P],
                            str_d[:, soff + c0 // 2 * P : soff + c1 // 2 * P],
                        )
                        nc.sync.dma_start(
                            yt4[0:D, c0 // 2 : c1 // 2, 0:W],
                            ytd_d[0:D, (toff + c0) // 2 * W : (toff + c1) // 2 * W],
                        )
                        nc.sync.dma_start(
                            yt4[D:P, c0 // 2 : c1 // 2, W:W2],
                            ytd_d[D:P, (toff + c0) // 2 * W : (toff + c1) // 2 * W],
                        )
                    # mask tail of the stream (only needed post-drain)
                    nc.sync.dma_start(
                        stb[:, nq * P : nq * P + nt * S],
                        str_d[:, soff + nq * P : soff + nq * P + nt * S],
                    )

                    # one matmul per tile-PAIR:
                    # Z[e, (h,r,j)] = us_{2q+h}[e] . Yt[v_j^{2q+h}][r]
                    # consecutive matmuls alternate PSUM banks so matmul i's
                    # PSUM drain can overlap matmul i+1's array fill
                    zh = zhpool.tile([P, TMAX * W], f16, tag="zh")
                    zh4 = zh[:].rearrange(
                        "p (u v x) -> p u v x", v=2, x=W2
                    )  # [p, pair-pair u, bank v, 80]
                    g0 = 0
                    while g0 < nq:
                        gp = min(GP, nq - g0)
                        za = zpoolA.tile([P, GP // 2, W2], f32, tag="za")
                        zb = zpoolB.tile([P, GP // 2, W2], f32, tag="zb")
                        for k in range(gp):
                            q = g0 + k
                            z = za if k % 2 == 0 else zb
                            nc.tensor.matmul(
                                z[:, k // 2, :],
                                lhsT=stb[:, q * P : (q + 1) * P],
                                rhs=ytb[:, q * W2 : (q + 1) * W2],
                            )
                        gh = gp // 2
                        nc.scalar.copy(
                            zh4[:, g0 // 2 : g0 // 2 + gh, 0, :], za[:, 0:gh, :]
                        )
                        nc.scalar.copy(
                            zh4[:, g0 // 2 : g0 // 2 + gh, 1, :], zb[:, 0:gh, :]
                        )
                        g0 += gp

                    # one-hot slot mask zeroes the 7 garbage items, then a
                    # 3-level fp16 tree reduces the 8 slots -> true scores
                    mkb = stb[:, nq * P : nq * P + nt * S]
                    prod = ppool.tile([P, TMAX * W], f16, tag="pr")
                    mk_bc = (
                        mkb.rearrange("p (t o s) -> p t o s", t=nt, o=1)
                        .to_broadcast([P, nt, R, S])
                    )
                    nc.vector.tensor_mul(
                        prod[:, : nt * W].rearrange(
                            "p (t r s) -> p t r s", t=nt, r=R
                        ),
                        zh[:, : nt * W].rearrange("p (t r s) -> p t r s", t=nt, r=R),
                        mk_bc,
                    )
                    v = prod[:, : nt * W].rearrange("p (t r s) -> p t r s", t=nt, r=R)
                    for w in (4, 2):
                        s = ppool.tile([P, TMAX * R * w], f16, tag=f"ps{w}")
                        nv = s[:, : nt * R * w].rearrange(
                            "p (t r s) -> p t r s", t=nt, r=R
                        )
                        nc.vector.tensor_add(
                            nv, v[:, :, :, 0:w], v[:, :, :, w : 2 * w]
                        )
                        v = nv
                    scores = spool.tile([P, TMAX * R], f16, tag="sc")
                    nc.vector.tensor_add(
                        scores[:, : nt * R].rearrange("p (t r) -> p t r", r=R),
                        v[:, :, :, 0],
                        v[:, :, :, 1],
                    )

                    # block tail: softmax-weighted rating, exp in f32 for
                    # range (|s| can reach ~48). num and den use the SAME
                    # exp values so their rounding errors cancel in the ratio.
                    exps = tpool.tile([P, TMAX * R], f32, tag="ex")
                    nc.scalar.activation(
                        exps[:, : nt * R],
                        scores[:, : nt * R],
                        mybir.ActivationFunctionType.Exp,
                    )
                    den = tpool.tile([P, TMAX], f32, tag="den")
                    nc.vector.tensor_reduce(
                        out=den[:, :nt],
                        in_=exps[:, : nt * R].rearrange("p (t r) -> p t r", r=R),
                        axis=X,
                        op=ADD,
                    )
                    nums = tpool.tile([P, TMAX * R], f32, tag="nums")
                    vals_bc = (
                        vals_t[:]
                        .rearrange("p (o r) -> p o r", o=1)
                        .to_broadcast([P, nt, R])
                    )
                    # gpsimd is near-idle; its tensor_mul keeps DVE free. The
                    # last block stays on DVE: its tail is the drain critical
                    # path and a gpsimd mul has ~2us latency.
                    last = bi == len(BLOCK_TILES) - 1
                    tail_eng = nc.vector if last else nc.gpsimd
                    tail_eng.tensor_mul(
                        nums[:, : nt * R].rearrange("p (t r) -> p t r", r=R),
                        exps[:, : nt * R].rearrange("p (t r) -> p t r", r=R),
                        vals_bc,
                    )
                    num = tpool.tile([P, TMAX], f32, tag="num")
                    nc.vector.tensor_reduce(
                        out=num[:, :nt],
                        in_=nums[:, : nt * R].rearrange("p (t r) -> p t r", r=R),
                        axis=X,
                        op=ADD,
                    )
                    rden = tpool.tile([P, TMAX], f32, tag="rden")
                    nc.vector.reciprocal_approx_fast(rden[:, :nt], den[:, :nt])
                    rat = tpool.tile([P, TMAX], f32, tag="rat")
                    tail_eng.tensor_mul(rat[:, :nt], num[:, :nt], rden[:, :nt])
                    nc.sync.dma_start(out_d[:, toff : toff + nt], rat[:, :nt])
                    toff += nt
                    soff += nq * P + nt * S
    nc.compile()
    return nc


def _pack_core(dst_sorted, n_tiles):
    """Greedy-pack an item-sorted edge run into 128-edge tiles with <= S
    distinct items per tile.

    Returns (edge_pos, slot_items, slot_of_edge):
      edge_pos[i]  = tile*128+p position of input edge i
      slot_items[t, j] = item id in slot j of tile t (0-padded)
      slot_of_edge[tile*128+p] = slot index (-1 for padding positions)
    """
    n = len(dst_sorted)
    # item run boundaries
    change = np.flatnonzero(np.diff(dst_sorted)) + 1
    starts = np.concatenate(([0], change))
    ends = np.concatenate((change, [n]))
    items = dst_sorted[starts]

    edge_pos = np.empty(n, np.int64)
    slot_items = np.zeros((n_tiles, S), np.int32)
    slot_of_edge = np.full(n_tiles * P, -1, np.int8)
    t, c, s = 0, 0, 0  # tile, edges in tile, slots used
    for item, a, b in zip(items, starts, ends):
        left = b - a
        while left > 0:
            if s == S or c == P:  # close tile (pad remainder)
                t += 1
                c, s = 0, 0
            take = min(left, P - c)
            pos = t * P + c
            edge_pos[b - left : b - left + take] = np.arange(pos, pos + take)
            slot_items[t, s] = item
            slot_of_edge[pos : pos + take] = s
            c += take
            s += 1  # continuation in next tile gets a fresh slot
            left -= take
    assert t < n_tiles, f"packing overflow: {t + 1} > {n_tiles}"
    return edge_pos, slot_items, slot_of_edge


def _prepare(ufeat, ifeat, Ps, src, dst):
    uf16 = ufeat.astype(np.float16)
    # Yt[v, r, d] = sum_f Ps[r, d, f] * ifeat[v, f]  (f32 gemm, then fp16)
    PT = np.ascontiguousarray(Ps.transpose(2, 0, 1).reshape(D, R * D))
    Yt = (ifeat @ PT).astype(np.float16).reshape(N_ITEMS, R, D)
    vals = np.tile(np.arange(1.0, 6.0, dtype=np.float32), (P, 1))

    order = np.argsort(dst, kind="stable")
    in_maps, metas = [], []
    for c in range(N_CORES):
        lo, hi = c * E_CORE, (c + 1) * E_CORE
        idx = order[lo:hi]  # original edge ids, item-sorted
        edge_pos, slot_items, slot_of_edge = _pack_core(dst[idx], N_TILES)

        # gathered user features, tile-pair stationary: even tile rows 0:63,
        # odd tile rows 64:127 -> (P, N_TILES/2*P)
        ug = np.zeros((PAD_E, D), np.float16)
        ug[edge_pos] = uf16[src[idx]]
        ust = np.ascontiguousarray(
            ug.reshape(N_TILES // 2, 2, P, D).transpose(1, 3, 0, 2)
        ).reshape(P, N_TILES // 2 * P)

        # per-tile Yt slot packs, pair-split: top rows = even tiles' packs,
        # bottom rows = odd tiles' packs -> (P, N_TILES/2*W), n = r*S+j
        g = Yt[slot_items]  # (NT, S, R, D)
        ytd = np.ascontiguousarray(
            g.reshape(N_TILES // 2, 2, S, R, D).transpose(1, 4, 0, 3, 2)
        ).reshape(P, N_TILES // 2 * W)

        # one-hot slot masks -> (P, N_TILES*S); padding rows all-zero
        oh = (
            slot_of_edge[:, None] == np.arange(S, dtype=np.int8)[None, :]
        ).astype(np.float16)
        msk = np.ascontiguousarray(
            oh.reshape(N_TILES, P, S).transpose(1, 0, 2)
        ).reshape(P, N_TILES * S)

        # fused per-block [usT | mask] stream
        pieces = []
        toff = 0
        for nt in BLOCK_TILES:
            pieces.append(ust[:, toff // 2 * P : (toff + nt) // 2 * P])
            pieces.append(msk[:, toff * S : (toff + nt) * S])
            toff += nt
        stream = np.ascontiguousarray(np.concatenate(pieces, axis=1))

        in_maps.append({"stream": stream, "ytd": ytd, "vals": vals})
        metas.append((idx, edge_pos))
    return in_maps, metas


def _install_profile_hook():
    """Make antenv.axon_hooks available so run_bass_kernel_spmd(trace=True)
    can capture NTFF profiles through the axon .so (used by test.py only)."""
    import types

    try:
        from antenv.axon_hooks import get_axon_ntff_profile_hook  # noqa: F401

        return
    except ImportError:
        pass
    import antenv
    from trn_agent_boot.trn_boot import _ntff_profile_via_ctypes

    hook = _ntff_profile_via_ctypes("/opt/axon/libaxon_pjrt.so")
    mod = types.ModuleType("antenv.axon_hooks")
    mod._hook = hook
    mod.get_axon_ntff_profile_hook = lambda: mod._hook
    mod.set_axon_ntff_profile_hook = lambda h: setattr(mod, "_hook", h)
    sys.modules["antenv.axon_hooks"] = mod
    antenv.axon_hooks = mod


def kernel(ufeat, ifeat, Ps, src, dst):
    from concourse.bass_utils import run_bass_kernel_spmd

    ufeat = np.asarray(ufeat, np.float32)
    ifeat = np.asarray(ifeat, np.float32)
    Ps = np.asarray(Ps, np.float32)
    src = np.asarray(src, np.int32)
    dst = np.asarray(dst, np.int32)

    if "nc" not in _NC_CACHE:
        _NC_CACHE["nc"] = _build_kernel()
    nc = _NC_CACHE["nc"]
    in_maps, metas = _prepare(ufeat, ifeat, Ps, src, dst)
    res = run_bass_kernel_spmd(nc, in_maps, core_ids=list(range(N_CORES)))
    out = np.zeros(E, np.float32)
    for c in range(N_CORES):
        o = res.results[c]["out"]  # [P, N_TILES]
        flat = o.T.reshape(-1)  # position = tile*128 + p
        idx, edge_pos = metas[c]
        out[idx] = flat[edge_pos]
    return out
